# revision 1
# baseline (speedup 1.0000x reference)
"""Distributed Trainium2 kernel for the ACloss loss function.

Shards the batch dim (16 -> 2 images/core) across 8 NeuronCores. Each core
streams its two images' heatmaps (out+tgt) through SBUF in two halves:
image b0 fully first, then image b1, so b0's argmax/coords/angle chain
runs hidden under b1's streaming and only b1's short chain remains
exposed at the end.

Engine split per core:
  - DMA descgen is parallelized across the two HW DGE queues: out-group
    waves (plus the small first/last tgt waves) issue from Sync, the big
    tgt waves from Scalar. Consts go via GpSimd's software DGE so data
    descgen starts right after the framework preamble.
  - DVE does the colmax reduces plus the small argmax/coords/norm chains.
  - GpSimd does all l2 subtracts, the indirect row gathers, and image
    b0's acos polynomial (emitted after half-1's subtracts so it fills
    the gap while b1's offsets are still in flight).
  - ActE does the l2 squares (accum) and the tail sqrts; its queue is
    ordered [h0 descgen, h1 descgen, squares, tail] so descriptor
    generation is never starved behind compute.
  - PE does the tiny transposes and outer-product matmuls.
Per-image partials (l2 cols, angle/dist sums) are combined by one PE
ones-matmul; the host sums the 8 cores and applies the final scalar math.
"""

import os
import numpy as np

B, L, H, W = 16, 19, 256, 256
NCORES = 8
B_LOC = B // NCORES            # 2 images per core
NH = 2 * L                     # 38 heatmaps per image (out l0..18 | tgt l0..18)
P = 128                        # partitions per heatmap tile
F = (H * W) // P               # 512 free elems per partition

_CACHE = {}
LAST_RESULTS = None

# DMA / reduce / l2 waves (landmark ranges) per half: small first chunk for
# an early compute start, tiny last chunk to shorten the colmax tail.
CH = [(0, 2), (2, 4), (6, 4), (10, 4), (14, 4), (18, 1)]
NW = len(CH)

A0, A1, A2, A3 = 1.5707288, -0.2121144, 0.0742610, -0.0187293


def _build():
    from contextlib import ExitStack

    import concourse.bass as bass
    import concourse.tile as tile
    from concourse import bacc, mybir

    fp32 = mybir.dt.float32
    i32 = mybir.dt.int32
    u32 = mybir.dt.uint32
    Alu = mybir.AluOpType
    Act = mybir.ActivationFunctionType
    AX = mybir.AxisListType

    nc = bacc.Bacc("TRN2", target_bir_lowering=False, debug=False,
                   num_devices=NCORES)

    data_p = nc.declare_dram_parameter("data", [2, B_LOC, L, H, W], fp32,
                                       isOutput=False)
    pri_p = nc.declare_dram_parameter("pri", [NH, P], fp32, isOutput=False)
    r0c_p = nc.declare_dram_parameter("r0c", [NH, 2], fp32, isOutput=False)
    ones_p = nc.declare_dram_parameter("onesv", [P, 1], fp32, isOutput=False)
    ident_p = nc.declare_dram_parameter("ident", [P, P], fp32, isOutput=False)
    res_p = nc.declare_dram_parameter("res", [8], fp32, isOutput=True)

    # [s, b, 128, l, 512] views: partition p holds rows {2p, 2p+1}
    dv = data_p.ap().rearrange("s b l (p h2) w -> s b p l (h2 w)", p=P, h2=2)
    # one flat row view over both sources for the indirect gathers
    all_flat = data_p.ap().rearrange("s b l (p h2) w -> (s b l p) (h2 w)",
                                     p=P, h2=2)

    with tile.TileContext(nc) as tc, ExitStack() as ctx:
        data = ctx.enter_context(tc.tile_pool(name="data", bufs=1))
        small = ctx.enter_context(tc.tile_pool(name="small", bufs=1))
        dpool = ctx.enter_context(tc.tile_pool(name="dpool", bufs=3))
        d2pool = ctx.enter_context(tc.tile_pool(name="d2pool", bufs=1))
        psum = ctx.enter_context(tc.tile_pool(name="psum", bufs=1, space="PSUM"))

        # constants via GpSimd's software DGE (HW DGE queues stay on data)
        pri = small.tile([NH, P], fp32, tag="pri")
        r0c = small.tile([NH, 2], fp32, tag="r0c")
        ones = small.tile([P, 1], fp32, tag="ones")
        ident = small.tile([P, P], fp32, tag="ident")
        nc.gpsimd.dma_start(out=pri[:], in_=pri_p[:])
        nc.gpsimd.dma_start(out=r0c[:], in_=r0c_p[:])
        nc.gpsimd.dma_start(out=ones[:], in_=ones_p[:])
        nc.gpsimd.dma_start(out=ident[:], in_=ident_p[:])

        # grp[b][s]: image b's heatmaps for source s (out/tgt). Separate
        # buffers per half — h1's DMAs must not WAR-wait on h0's reduces.
        grp = [[data.tile([P, L, F], fp32, tag=f"grp{b}{s}",
                          name=f"grp{b}{s}") for s in range(2)]
               for b in range(B_LOC)]
        # colmax columns: img*38 + (src*19 + l)
        colmax = small.tile([P, 2 * NH], fp32, tag="colmax")
        l2cols = small.tile([P, 2 * NW], fp32, tag="l2cols")
        sums19 = small.tile([L, 4], fp32, tag="sums19")

        st = {}  # per-image chain state

        def emit_dma(gi):
            b, ci = divmod(gi, NW)
            lo, nl = CH[ci]
            nc.sync.dma_start(out=grp[b][0][:, lo:lo + nl, :],
                              in_=dv[0, b, :, lo:lo + nl, :])
            nc.scalar.dma_start(out=grp[b][1][:, lo:lo + nl, :],
                                in_=dv[1, b, :, lo:lo + nl, :])

        def emit_compute(gi):
            b, ci = divmod(gi, NW)
            lo, nl = CH[ci]
            for s in range(2):
                nc.vector.tensor_reduce(
                    out=colmax[:, b * NH + s * L + lo:
                               b * NH + s * L + lo + nl],
                    in_=grp[b][s][:, lo:lo + nl, :],
                    axis=AX.X, op=Alu.max)
            d = dpool.tile([P, 4, F], fp32, tag="d", name=f"d{gi}")
            st[gi] = d
            nc.gpsimd.tensor_tensor(out=d[:, 0:nl, :],
                                    in0=grp[b][0][:, lo:lo + nl, :],
                                    in1=grp[b][1][:, lo:lo + nl, :],
                                    op=Alu.subtract)

        def emit_square(gi):
            # l2 square+accum for global wave gi, emitted three waves late
            # so it never stalls the ring-paced descgen on the ActE queue
            b, ci = divmod(gi, NW)
            lo, nl = CH[ci]
            d = st[gi]
            d2 = d2pool.tile([P, 4, F], fp32, tag="d2", name=f"d2{gi}")
            nc.scalar.activation(out=d2[:, 0:nl, :], in_=d[:, 0:nl, :],
                                 func=Act.Square,
                                 accum_out=l2cols[:, gi:gi + 1])

        def chain_pre(b):
            # winning-partition selection per heatmap (DVE + one PE transpose)
            base = b * NH
            colmaxT = psum.tile([NH, P], fp32, tag="colmaxT", space="PSUM",
                                name=f"colmaxT{b}")
            nc.tensor.transpose(out=colmaxT[:], in_=colmax[:, base:base + NH],
                                identity=ident[:])
            gmax = small.tile([NH, 1], fp32, tag="gmax", name=f"gmax{b}")
            nc.vector.tensor_reduce(out=gmax[:], in_=colmaxT[:], axis=AX.X,
                                    op=Alu.max)
            # tmp[h,p] = (colmax_p==gmax_h)*(128-p); max -> 128 - wp_first
            tmp = small.tile([NH, P], fp32, tag="tmpw", name=f"tmpw{b}")
            nc.vector.scalar_tensor_tensor(out=tmp[:], in0=colmaxT[:],
                                           scalar=gmax[:, 0:1], in1=pri[:],
                                           op0=Alu.is_ge, op1=Alu.mult)
            wsel = small.tile([NH, 1], fp32, tag="wsel", name=f"wsel{b}")
            nc.vector.tensor_reduce(out=wsel[:], in_=tmp[:], axis=AX.X,
                                    op=Alu.max)
            # dram row = ((s*2+b)*19+l)*128 + wp = r0c - wsel  (r0c bakes +128)
            offs_i = small.tile([NH, 1], i32, tag="offs_i", name=f"offs{b}")
            nc.vector.scalar_tensor_tensor(out=offs_i[:], in0=wsel[:],
                                           scalar=-1.0, in1=r0c[:, b:b + 1],
                                           op0=Alu.mult, op1=Alu.add)
            st[f"wsel{b}"] = wsel
            st[f"offs{b}"] = offs_i

        def chain_gather(b):
            rows = small.tile([NH, F], fp32, tag="rows", name=f"rows{b}")
            nc.gpsimd.indirect_dma_start(
                out=rows[:], out_offset=None, in_=all_flat[:],
                in_offset=bass.IndirectOffsetOnAxis(ap=st[f"offs{b}"][:, 0:1],
                                                    axis=0))
            st[f"rows{b}"] = rows

        def chain_post(b):
            # in-row argmax + coords + guarded 1/norm (DVE), v/nsq
            # transposes (PE), psum->sbuf copies + norm sqrt (ActE)
            rows, wsel = st[f"rows{b}"], st[f"wsel{b}"]
            max8 = small.tile([NH, 8], fp32, tag="max8", name=f"max8{b}")
            nc.vector.max(out=max8[:], in_=rows[:])
            idx8 = small.tile([NH, 8], u32, tag="idx8", name=f"idx8{b}")
            nc.vector.max_index(out=idx8[:], in_max=max8[:], in_values=rows[:])
            widx = small.tile([NH, 1], fp32, tag="widx", name=f"widx{b}")
            nc.vector.tensor_copy(out=widx[:], in_=idx8[:, 0:1])

            # coords: y = 2*wp + (widx>=256), x = widx - 256*(widx>=256)
            # v = coords - 128; wp = 128 - wsel
            thi = small.tile([NH, 1], fp32, tag="thi", name=f"thi{b}")
            nc.vector.tensor_single_scalar(out=thi[:], in_=widx[:],
                                           scalar=256.0, op=Alu.is_ge)
            vc = small.tile([NH, 2], fp32, tag="vc", name=f"vc{b}")
            vyt = small.tile([NH, 1], fp32, tag="vyt", name=f"vyt{b}")
            nc.vector.scalar_tensor_tensor(out=vyt[:], in0=wsel[:],
                                           scalar=-2.0, in1=thi[:],
                                           op0=Alu.mult, op1=Alu.add)
            nc.vector.tensor_single_scalar(out=vc[:, 0:1], in_=vyt[:],
                                           scalar=128.0, op=Alu.add)
            vxt = small.tile([NH, 1], fp32, tag="vxt", name=f"vxt{b}")
            nc.vector.scalar_tensor_tensor(out=vxt[:], in0=thi[:],
                                           scalar=-256.0, in1=widx[:],
                                           op0=Alu.mult, op1=Alu.add)
            nc.vector.tensor_single_scalar(out=vc[:, 1:2], in_=vxt[:],
                                           scalar=-128.0, op=Alu.add)

            vsq = small.tile([NH, 2], fp32, tag="vsq", name=f"vsq{b}")
            nc.vector.tensor_tensor(out=vsq[:], in0=vc[:], in1=vc[:],
                                    op=Alu.mult)
            nsqc = small.tile([NH, 1], fp32, tag="nsqc", name=f"nsqc{b}")
            nc.vector.tensor_reduce(out=nsqc[:], in_=vsq[:], axis=AX.X,
                                    op=Alu.add)
            v2p = psum.tile([2, NH], fp32, tag="v2p", space="PSUM",
                            name=f"v2p{b}")
            nc.tensor.transpose(out=v2p[:], in_=vc[:],
                                identity=ident[0:NH, 0:NH])
            v2 = small.tile([2, NH], fp32, tag=f"v2_{b}")
            nc.scalar.copy(out=v2[:], in_=v2p[:])
            nsqp = psum.tile([1, NH], fp32, tag="nsqp", space="PSUM",
                             name=f"nsqp{b}")
            nc.tensor.transpose(out=nsqp[:], in_=nsqc[:],
                                identity=ident[0:NH, 0:NH])
            nsq = small.tile([1, NH], fp32, tag=f"nsq_{b}")
            nc.scalar.copy(out=nsq[:], in_=nsqp[:])

            # guarded 1/norm and nonzero mask (all on partition 0)
            nrm = small.tile([1, NH], fp32, tag="nrm", name=f"nrm{b}")
            nc.scalar.activation(out=nrm[:], in_=nsq[:], func=Act.Sqrt)
            zed = small.tile([1, NH], fp32, tag="zed", name=f"zed{b}")
            nc.vector.tensor_single_scalar(out=zed[:], in_=nsq[:], scalar=0.0,
                                           op=Alu.is_le)
            nzm = small.tile([1, NH], fp32, tag="nzm", name=f"nzm{b}")
            nc.vector.tensor_scalar(out=nzm[:], in0=zed[:], scalar1=-1.0,
                                    scalar2=1.0, op0=Alu.mult, op1=Alu.add)
            nsafe = small.tile([1, NH], fp32, tag="nsafe", name=f"nsafe{b}")
            nc.vector.tensor_tensor(out=nsafe[:], in0=nrm[:], in1=zed[:],
                                    op=Alu.add)
            rec = small.tile([1, NH], fp32, tag="rec", name=f"rec{b}")
            nc.vector.reciprocal(out=rec[:], in_=nsafe[:])
            rr = small.tile([1, NH], fp32, tag=f"rr_{b}")
            nc.vector.tensor_tensor(out=rr[:], in0=rec[:], in1=nzm[:],
                                    op=Alu.mult)
            st[f"v2_{b}"] = v2
            st[f"nsq_{b}"] = nsq
            st[f"rr_{b}"] = rr

        def batched_tail():
            # outer-product matmuls for both images into [19, 76] psum
            W2 = 2 * NH
            onesrow = small.tile([1, NH], fp32, tag="onesrow")
            nc.vector.memset(onesrow[:], 1.0)
            dots = psum.tile([L, W2], fp32, tag="dots", space="PSUM")
            rrP = psum.tile([L, W2], fp32, tag="rrP", space="PSUM")
            osP = psum.tile([L, W2], fp32, tag="osP", space="PSUM")
            for b in range(B_LOC):
                v2, nsq, rr = st[f"v2_{b}"], st[f"nsq_{b}"], st[f"rr_{b}"]
                for s in range(2):
                    sl = slice(s * L, (s + 1) * L)
                    osl = slice(b * NH + s * L, b * NH + (s + 1) * L)
                    nc.tensor.matmul(out=dots[:, osl], lhsT=v2[:, sl],
                                     rhs=v2[:, sl], start=True, stop=True)
                    nc.tensor.matmul(out=rrP[:, osl], lhsT=rr[0:1, sl],
                                     rhs=rr[0:1, sl], start=True, stop=True)
                    nc.tensor.matmul(out=osP[:, osl], lhsT=nsq[0:1, sl],
                                     rhs=onesrow[0:1, sl], start=True,
                                     stop=False)
                    nc.tensor.matmul(out=osP[:, osl], lhsT=onesrow[0:1, sl],
                                     rhs=nsq[0:1, sl], start=False, stop=True)

            # angle via the A&S 4.4.45 polynomial, batched over both images:
            #   acos(x) = sqrt(1-x)*(a0 + a1 x + a2 x^2 + a3 x^3), x in [0,1]
            #   acos(x<0) = pi - acos(-x);  abs err <= 5e-5
            # nz mask = (rrP > 0), true iff both landmarks are nonzero
            ve = nc.vector
            dotsS = small.tile([L, W2], fp32, tag="dotsS")
            nc.scalar.copy(out=dotsS[:], in_=dots[:])
            msk = small.tile([L, W2], fp32, tag="msk")
            ve.tensor_single_scalar(out=msk[:], in_=rrP[:], scalar=0.0,
                                    op=Alu.is_gt)
            cosm = small.tile([L, W2], fp32, tag="cosm")
            ve.tensor_tensor(out=cosm[:], in0=dotsS[:], in1=rrP[:],
                             op=Alu.mult)
            mng = small.tile([L, W2], fp32, tag="mng")
            ve.tensor_single_scalar(out=mng[:], in_=cosm[:], scalar=0.0,
                                    op=Alu.is_lt)
            flp = small.tile([L, W2], fp32, tag="flp")
            ve.tensor_scalar(out=flp[:], in0=mng[:], scalar1=-2.0,
                             scalar2=1.0, op0=Alu.mult, op1=Alu.add)
            ax = small.tile([L, W2], fp32, tag="ax")
            ve.tensor_tensor(out=ax[:], in0=cosm[:], in1=flp[:], op=Alu.mult)
            ve.tensor_single_scalar(out=ax[:], in_=ax[:], scalar=1.0,
                                    op=Alu.min)
            h1 = small.tile([L, W2], fp32, tag="h1")
            ve.tensor_scalar(out=h1[:], in0=ax[:], scalar1=A3,
                             scalar2=A2, op0=Alu.mult, op1=Alu.add)
            h2 = small.tile([L, W2], fp32, tag="h2")
            ve.tensor_tensor(out=h2[:], in0=h1[:], in1=ax[:], op=Alu.mult)
            ve.tensor_single_scalar(out=h2[:], in_=h2[:], scalar=A1,
                                    op=Alu.add)
            h3 = small.tile([L, W2], fp32, tag="h3")
            ve.tensor_tensor(out=h3[:], in0=h2[:], in1=ax[:], op=Alu.mult)
            ve.tensor_single_scalar(out=h3[:], in_=h3[:], scalar=A0,
                                    op=Alu.add)
            qq = small.tile([L, W2], fp32, tag="qq")
            ve.tensor_scalar(out=qq[:], in0=ax[:], scalar1=-1.0,
                             scalar2=1.0, op0=Alu.mult, op1=Alu.add)
            sq = small.tile([L, W2], fp32, tag="sq")
            nc.scalar.activation(out=sq[:], in_=qq[:], func=Act.Sqrt)
            acp = small.tile([L, W2], fp32, tag="acp")
            ve.tensor_tensor(out=acp[:], in0=sq[:], in1=h3[:], op=Alu.mult)
            ac2 = small.tile([L, W2], fp32, tag="ac2")
            ve.tensor_tensor(out=ac2[:], in0=acp[:], in1=flp[:], op=Alu.mult)
            ac3 = small.tile([L, W2], fp32, tag="ac3")
            ve.scalar_tensor_tensor(out=ac3[:], in0=mng[:],
                                    scalar=float(np.pi), in1=ac2[:],
                                    op0=Alu.mult, op1=Alu.add)
            ang = small.tile([L, W2], fp32, tag="ang")
            ve.tensor_tensor(out=ang[:], in0=ac3[:], in1=msk[:], op=Alu.mult)

            # dist = sqrt(max(osP - 2*dots, 0))
            d2m = small.tile([L, W2], fp32, tag="d2m")
            ve.scalar_tensor_tensor(out=d2m[:], in0=dotsS[:], scalar=-2.0,
                                    in1=osP[:], op0=Alu.mult, op1=Alu.add)
            ve.tensor_single_scalar(out=d2m[:], in_=d2m[:], scalar=0.0,
                                    op=Alu.max)
            dist = small.tile([L, W2], fp32, tag="dist")
            nc.scalar.activation(out=dist[:], in_=d2m[:], func=Act.Sqrt)

            # per-image |out - tgt| sums for angle and dist
            for b in range(B_LOC):
                for i, mat in enumerate((ang, dist)):
                    dtmp = small.tile([L, L], fp32, tag="dtmp",
                                      name=f"dtmp{b}_{i}")
                    nc.vector.tensor_tensor(
                        out=dtmp[:], in0=mat[:, b * NH:b * NH + L],
                        in1=mat[:, b * NH + L:(b + 1) * NH],
                        op=Alu.subtract)
                    nc.vector.tensor_reduce(
                        out=sums19[:, 2 * b + i:2 * b + i + 1],
                        in_=dtmp[:], axis=AX.X, op=Alu.add,
                        apply_absolute_value=True)

        # ---- emission order (per-engine queues are in-order) ----
        # ---- emission order: one global 12-wave stream; descgen runs
        # two waves ahead, squares three waves behind; b0's chain is
        # slotted into h1's stream ----
        NG = 2 * NW
        emit_dma(0)
        emit_dma(1)
        for gi in range(NG):
            emit_compute(gi)
            if gi + 2 < NG:
                emit_dma(gi + 2)
            if gi >= 3:
                emit_square(gi - 3)
            if gi == NW - 1:
                chain_pre(0)      # DVE: b0 winning-partition chain
                chain_gather(0)   # GpSimd: b0 row gather (between subs)
            if gi == NW + 1:
                chain_post(0)     # DVE/ActE slot while h1 streams
        for gi in range(NG - 3, NG):
            emit_square(gi)
        chain_pre(1)
        chain_gather(1)
        chain_post(1)
        batched_tail()

        # ---- final partition reductions via one PE ones-matmul ----
        combo = small.tile([P, 5], fp32, tag="combo")
        nc.vector.memset(combo[:], 0.0)
        nc.vector.tensor_reduce(out=combo[:, 0:1], in_=l2cols[:],
                                axis=AX.X, op=Alu.add)
        nc.vector.tensor_copy(out=combo[0:L, 1:5], in_=sums19[:])
        finP = psum.tile([5, 1], fp32, tag="finP", space="PSUM")
        nc.tensor.matmul(out=finP[:], lhsT=combo[:], rhs=ones[:],
                         start=True, stop=True)
        finsb = small.tile([5, 1], fp32, tag="finsb")
        nc.scalar.copy(out=finsb[:], in_=finP[:])
        nc.sync.dma_start(out=res_p[0:5], in_=finsb[:])

    nc.compile()
    return nc


def _consts():
    pri = np.broadcast_to((P - np.arange(P, dtype=np.float32))[None, :],
                          (NH, P)).copy()
    # r0c[h, b]: dram row base (+128) of heatmap (b, s, l), h = s*19+l,
    # within the [2, B_LOC, L, 128, 512] row view
    r0c = np.empty((NH, 2), dtype=np.float32)
    for bb in range(B_LOC):
        for s in range(2):
            for l in range(L):
                r0c[s * L + l, bb] = ((s * B_LOC + bb) * L + l) * P + P
    ones = np.ones((P, 1), dtype=np.float32)
    ident = np.eye(P, dtype=np.float32)
    return {"pri": pri, "r0c": r0c, "onesv": ones, "ident": ident}


def kernel(output: np.ndarray, target: np.ndarray) -> np.ndarray:
    global LAST_RESULTS
    from concourse.bass_utils import run_bass_kernel_spmd

    if "nc" not in _CACHE:
        _CACHE["nc"] = _build()
    nc = _CACHE["nc"]

    output = np.ascontiguousarray(output, dtype=np.float32)
    target = np.ascontiguousarray(target, dtype=np.float32)
    consts = _consts()
    in_maps = []
    for c in range(NCORES):
        m = {"data": np.stack([output[c * B_LOC:(c + 1) * B_LOC],
                               target[c * B_LOC:(c + 1) * B_LOC]])}
        m.update(consts)
        in_maps.append(m)

    trace = os.environ.get("KERNEL_TRACE") == "1"
    res = run_bass_kernel_spmd(nc, in_maps, list(range(NCORES)), trace=trace)
    LAST_RESULTS = res

    l2_sum = 0.0
    ang_sum = 0.0
    dist_sum = 0.0
    for c in range(NCORES):
        r = np.asarray(res.results[c]["res"], dtype=np.float64).reshape(-1)
        l2_sum += r[0]
        ang_sum += (r[1] + r[3]) / (L * L)
        dist_sum += (r[2] + r[4]) / (L * L)

    l2 = l2_sum / (B * L * H * W)
    w = 1.0 + ang_sum + np.log(dist_sum + 1e-10)
    loss = l2 * w
    return np.array([loss, l2, w, ang_sum, dist_sum], dtype=np.float32)

